# revision 1
# baseline (speedup 1.0000x reference)
"""KgAdapterCrossAttention kernel for 8 trn2 NeuronCores.

Sharding: core = (batch b, query-half qh).  Each core computes attention for
1024 queries of one batch element against all 2048 keys.

Layout strategy (all transposes done on host, layout-only — all FLOPs on
device):
  - activations passed d-major (xqT [256, NQ], xkT [256, NK]) so QKV
    projections and the S^T matmul need no on-device transpose,
  - scores computed transposed S^T [k, q], which matches align_mask's
    natural (K, Q) layout — no mask transpose,
  - softmax without max-subtraction (scores are ~N(0,1); exp is safe) so no
    cross-partition max is needed; the denominator comes for free from a
    ones-column appended to V,
  - attention output A [q, 65] per head accumulates over k-tiles in PSUM with
    P~^T tiles as the stationary operand; per-head normalize is a native
    per-partition scalar multiply,
  - final O projection after a cheap 128x128 PE transpose of A.
"""

import os
import sys

import numpy as np

try:
    import concourse.bass as bass
except ImportError:
    for _p in ("/opt/trn_rl_repo", os.path.expanduser("~/.axon_site/_ro/trn_rl_repo")):
        if os.path.isdir(_p) and _p not in sys.path:
            sys.path.insert(0, _p)
    import concourse.bass as bass

import concourse.mybir as mybir
import concourse.tile as tile
from concourse import bacc
from concourse.masks import make_identity
from contextlib import ExitStack

F32 = mybir.dt.float32
EXP = mybir.ActivationFunctionType.Exp

P = 128
HID = 256
NHEAD = 4
DHEAD = 64
NQ = 1024  # queries per core
NK = 2048  # keys (full)
QBLK = 256
NQB = NQ // QBLK  # 4
NKT = NK // P  # 16
NCT = HID // P  # 2 contraction tiles over hidden


def build(with_attn_mask: bool) -> bass.Bass:
    nc = bacc.Bacc()
    xqT = nc.declare_dram_parameter("xqT", [HID, NQ], F32, isOutput=False)
    xkT = nc.declare_dram_parameter("xkT", [HID, NK], F32, isOutput=False)
    amf = nc.declare_dram_parameter("amf", [NK, NQ], F32, isOutput=False)
    wqT = nc.declare_dram_parameter("wqT", [HID, HID], F32, isOutput=False)
    wkT = nc.declare_dram_parameter("wkT", [HID, HID], F32, isOutput=False)
    wvT = nc.declare_dram_parameter("wvT", [HID, HID], F32, isOutput=False)
    woT = nc.declare_dram_parameter("woT", [HID, HID], F32, isOutput=False)
    amk = None
    if with_attn_mask:
        amk = nc.declare_dram_parameter("amk", [NK, NQ], F32, isOutput=False)
    out_d = nc.declare_dram_parameter("out", [NQ, HID], F32, isOutput=True)

    with tile.TileContext(nc) as tc, ExitStack() as ctx:
        const = ctx.enter_context(tc.tile_pool(name="const", bufs=1))
        big = ctx.enter_context(tc.tile_pool(name="big", bufs=1))
        ptp = ctx.enter_context(tc.tile_pool(name="ptp", bufs=1))
        amp = ctx.enter_context(tc.tile_pool(name="amp", bufs=2))
        wrk = ctx.enter_context(tc.tile_pool(name="wrk", bufs=3))
        outp = ctx.enter_context(tc.tile_pool(name="outp", bufs=3))
        ps_st = ctx.enter_context(tc.tile_pool(name="ps_st", bufs=3, space="PSUM"))
        ps_a = ctx.enter_context(tc.tile_pool(name="ps_a", bufs=2, space="PSUM"))
        ps_t = ctx.enter_context(tc.tile_pool(name="ps_t", bufs=2, space="PSUM"))
        ps_o = ctx.enter_context(tc.tile_pool(name="ps_o", bufs=1, space="PSUM"))

        # --- load weights + activations ---
        def load2(name, src, width):
            ts = []
            for t in range(2):
                tl = const.tile([P, width], F32, tag=f"{name}{t}", name=f"{name}{t}")
                nc.sync.dma_start(out=tl, in_=src[t * P : (t + 1) * P, :])
                ts.append(tl)
            return ts

        wq_sb = load2("wq", wqT, HID)
        wk_sb = load2("wk", wkT, HID)
        wv_sb = load2("wv", wvT, HID)
        wo_sb = load2("wo", woT, HID)
        xq_sb = []
        xk_sb = []
        for t in range(2):
            tl = big.tile([P, NQ], F32, tag=f"xq{t}", name=f"xq{t}")
            nc.sync.dma_start(out=tl, in_=xqT[t * P : (t + 1) * P, :])
            xq_sb.append(tl)
            tl = big.tile([P, NK], F32, tag=f"xk{t}", name=f"xk{t}")
            nc.sync.dma_start(out=tl, in_=xkT[t * P : (t + 1) * P, :])
            xk_sb.append(tl)

        ident = const.tile([P, P], F32, tag="ident", name="ident")
        make_identity(nc, ident)

        # --- projections ---
        # QT[o, q] = sum_i wqT[i, o] * xqT[i, q]   (wqT pre-scaled by 1/8)
        qt_sb = [big.tile([P, NQ], F32, tag=f"qt{t}", name=f"qt{t}") for t in range(2)]
        for t in range(2):
            for nb in range(NQ // 512):
                ps = ps_st.tile([P, 512], F32, tag="st", name="st")
                for ct in range(NCT):
                    nc.tensor.matmul(
                        ps,
                        lhsT=wq_sb[ct][:, t * P : (t + 1) * P],
                        rhs=xq_sb[ct][:, nb * 512 : (nb + 1) * 512],
                        start=(ct == 0),
                        stop=(ct == NCT - 1),
                    )
                nc.vector.tensor_copy(qt_sb[t][:, nb * 512 : (nb + 1) * 512], ps)

        kt_sb = [big.tile([P, NK], F32, tag=f"kt{t}", name=f"kt{t}") for t in range(2)]
        for t in range(2):
            for nb in range(NK // 512):
                ps = ps_st.tile([P, 512], F32, tag="st", name="st")
                for ct in range(NCT):
                    nc.tensor.matmul(
                        ps,
                        lhsT=wk_sb[ct][:, t * P : (t + 1) * P],
                        rhs=xk_sb[ct][:, nb * 512 : (nb + 1) * 512],
                        start=(ct == 0),
                        stop=(ct == NCT - 1),
                    )
                nc.vector.tensor_copy(kt_sb[t][:, nb * 512 : (nb + 1) * 512], ps)

        # V''[ktok, h, 0:64] = V rows; V''[ktok, h, 64] = 1.0 (denominator col)
        vpp = []
        for kt in range(NKT):
            tl = big.tile([P, NHEAD, DHEAD + 1], F32, tag=f"v{kt}", name=f"v{kt}")
            ps = ps_st.tile([P, HID], F32, tag="st", name="st")
            for ct in range(NCT):
                nc.tensor.matmul(
                    ps,
                    lhsT=xk_sb[ct][:, kt * P : (kt + 1) * P],
                    rhs=wv_sb[ct],
                    start=(ct == 0),
                    stop=(ct == NCT - 1),
                )
            nc.vector.tensor_copy(
                tl[:, :, 0:DHEAD], ps.rearrange("p (h d) -> p h d", h=NHEAD)
            )
            nc.vector.memset(tl[:, :, DHEAD : DHEAD + 1], 1.0)
            vpp.append(tl)

        amf_r = amf.rearrange("(t p) q -> p t q", p=P)
        amk_r = amk.rearrange("(t p) q -> p t q", p=P) if with_attn_mask else None

        # --- attention over q-blocks ---
        for qb in range(NQB):
            qsl = slice(qb * QBLK, (qb + 1) * QBLK)
            am_t = amp.tile([P, NKT, QBLK], F32, tag="am", name="am")
            nc.sync.dma_start(out=am_t, in_=amf_r[:, :, qsl])
            if with_attn_mask:
                amk_t = amp.tile([P, NKT, QBLK], F32, tag="amk", name="amk")
                nc.sync.dma_start(out=amk_t, in_=amk_r[:, :, qsl])
            pts = {}
            for h in range(NHEAD):
                t, po = h // 2, (h % 2) * DHEAD
                for kt in range(NKT):
                    stp = ps_st.tile([P, QBLK], F32, tag="st", name="st")
                    nc.tensor.matmul(
                        stp,
                        lhsT=kt_sb[t][po : po + DHEAD, kt * P : (kt + 1) * P],
                        rhs=qt_sb[t][po : po + DHEAD, qsl],
                        start=True,
                        stop=True,
                    )
                    pt = ptp.tile([P, QBLK], F32, tag=f"pt{h}_{kt}", name=f"pt{h}_{kt}")
                    if with_attn_mask:
                        tmp = wrk.tile([P, QBLK], F32, tag="masked", name="masked")
                        nc.vector.tensor_add(tmp, stp, amk_t[:, kt, :])
                        nc.scalar.activation(pt, tmp, EXP)
                    else:
                        nc.scalar.activation(pt, stp, EXP)
                    nc.vector.tensor_mul(pt, pt, am_t[:, kt, :])
                    pts[(h, kt)] = pt
            for qt in range(QBLK // P):
                anorm = wrk.tile([P, HID], F32, tag="anorm", name="anorm")
                for h in range(NHEAD):
                    ap_ = ps_a.tile([P, DHEAD + 1], F32, tag="a", name="a")
                    for kt in range(NKT):
                        nc.tensor.matmul(
                            ap_,
                            lhsT=pts[(h, kt)][:, qt * P : (qt + 1) * P],
                            rhs=vpp[kt][:, h, :],
                            start=(kt == 0),
                            stop=(kt == NKT - 1),
                        )
                    rec = wrk.tile([P, 1], F32, tag="rec", name="rec")
                    nc.vector.reciprocal(rec, ap_[:, DHEAD : DHEAD + 1])
                    nc.vector.tensor_scalar_mul(
                        anorm[:, h * DHEAD : (h + 1) * DHEAD], ap_[:, 0:DHEAD], rec
                    )
                o_ps = ps_o.tile([P, HID], F32, tag="o", name="o")
                for ct in range(NCT):
                    tp = ps_t.tile([P, P], F32, tag="t", name="t")
                    nc.tensor.transpose(tp, anorm[:, ct * P : (ct + 1) * P], ident)
                    att = wrk.tile([P, P], F32, tag=f"att{ct}", name=f"att{ct}")
                    nc.vector.tensor_copy(att, tp)
                    nc.tensor.matmul(
                        o_ps, lhsT=att, rhs=wo_sb[ct], start=(ct == 0), stop=(ct == NCT - 1)
                    )
                ob = outp.tile([P, HID], F32, tag="ob", name="ob")
                nc.vector.tensor_copy(ob, o_ps)
                q0 = qb * QBLK + qt * P
                nc.sync.dma_start(out=out_d[q0 : q0 + P, :], in_=ob)
    nc.compile()
    return nc


_NC_CACHE = {}
_last_in_maps = None


def _get_nc(with_attn_mask: bool) -> bass.Bass:
    if with_attn_mask not in _NC_CACHE:
        _NC_CACHE[with_attn_mask] = build(with_attn_mask)
    return _NC_CACHE[with_attn_mask]


def kernel(q_hidden_states, k_hidden_states, attention_mask, align_mask, Wq, Wk, Wv, Wo):
    from concourse.bass_utils import run_bass_kernel_spmd

    q_hidden_states = np.asarray(q_hidden_states, np.float32)
    k_hidden_states = np.asarray(k_hidden_states, np.float32)
    attention_mask = np.asarray(attention_mask, np.float32)
    align_mask = np.asarray(align_mask)
    B, Q, _ = q_hidden_states.shape
    qh_len = Q // 2  # 1024

    use_mask = bool(np.any(attention_mask))
    nc = _get_nc(use_mask)

    wq = np.ascontiguousarray(np.asarray(Wq, np.float32).T) / np.float32(8.0)
    wk = np.ascontiguousarray(np.asarray(Wk, np.float32).T)
    wv = np.ascontiguousarray(np.asarray(Wv, np.float32).T)
    wo = np.ascontiguousarray(np.asarray(Wo, np.float32).T)

    in_maps = []
    for core in range(8):
        b, qh = divmod(core, 2)
        qsl = slice(qh * qh_len, (qh + 1) * qh_len)
        m = {
            "xqT": np.ascontiguousarray(q_hidden_states[b, qsl].T),
            "xkT": np.ascontiguousarray(k_hidden_states[b].T),
            "amf": np.ascontiguousarray(align_mask[b, :, qsl].astype(np.float32)),
            "wqT": wq,
            "wkT": wk,
            "wvT": wv,
            "woT": wo,
        }
        if use_mask:
            m["amk"] = np.ascontiguousarray(attention_mask[b, 0, qsl, :].T)
        in_maps.append(m)

    global _last_in_maps
    _last_in_maps = in_maps
    res = run_bass_kernel_spmd(nc, in_maps, list(range(8))).results
    out = np.empty((B, Q, HID), np.float32)
    for core in range(8):
        b, qh = divmod(core, 2)
        out[b, qh * qh_len : (qh + 1) * qh_len] = res[core]["out"]
    return out



# revision 4
# speedup vs baseline: 2.3743x; 2.3743x over previous
"""KgAdapterCrossAttention kernel for 8 trn2 NeuronCores.

Sharding: core = (batch b, query-half qh). Each core: 1024 queries x 2048 keys,
4 heads, hidden 256.

Design (cost-model-driven):
  - All matmuls in bf16 (1 cycle/row on PE vs 4 for fp32).
  - Scores computed transposed S^T [k, q] per head (matches mask layout),
    softmax without max-subtraction, denominator via a ones-column in V.
  - Masking: host folds align_mask AND exp(attention_mask) into ONE bf16
    multiplier CM[k, q]; device does P = exp(S^T) * CM  (one DVE multiply).
  - exp on the ACT engine is the bottleneck (~8.4M elems/core); everything
    else (PE matmuls, DVE mask-mul, Pool psum->sbuf copies, DMA) is sized to
    hide underneath it. Pipeline unit = (q-block of 256, head).
  - PSUM budget (8 banks): scores 2x[128,4,256] (4) + A accum 2x[128,4,65]
    (2) + transpose/output 2x[128,512] (2).
  - PV accumulates A[q,65] for all 4 heads into one psum bank as a single
    merged accumulation group; per-head normalize is a per-partition scalar
    multiply; O projection after a 128x128 PE transpose of the normalized A.
  - Outputs written bf16, converted to f32 on host.
"""

import os
import sys

import numpy as np

try:
    import concourse.bass as bass
except ImportError:
    for _p in ("/opt/trn_rl_repo", os.path.expanduser("~/.axon_site/_ro/trn_rl_repo")):
        if os.path.isdir(_p) and _p not in sys.path:
            sys.path.insert(0, _p)
    import concourse.bass as bass

import ml_dtypes
import concourse.mybir as mybir
import concourse.tile as tile
from concourse import bacc
from concourse.masks import make_identity
from contextlib import ExitStack

F32 = mybir.dt.float32
BF16 = mybir.dt.bfloat16
EXP = mybir.ActivationFunctionType.Exp
BF = ml_dtypes.bfloat16

P = 128
HID = 256
NHEAD = 4
DHEAD = 64
NQ = 1024  # queries per core
NK = 2048  # keys (full)
QC = 256  # query block
NQC = NQ // QC  # 4
NKT = NK // P  # 16
KTG = 4  # k-tiles per exp group


def build() -> bass.Bass:
    nc = bacc.Bacc()
    xq_d = nc.declare_dram_parameter("xq", [P, 2, NQ], BF16, isOutput=False)
    xk_d = nc.declare_dram_parameter("xk", [P, 2, NK], BF16, isOutput=False)
    w_d = nc.declare_dram_parameter("w", [P, 8, HID], BF16, isOutput=False)
    cm_d = nc.declare_dram_parameter("cm", [NQC, P, NKT, QC], BF16, isOutput=False)
    out_d = nc.declare_dram_parameter("out", [NQ, HID], BF16, isOutput=True)

    with tile.TileContext(nc) as tc, ExitStack() as ctx:
        const = ctx.enter_context(tc.tile_pool(name="const", bufs=1))
        big = ctx.enter_context(tc.tile_pool(name="big", bufs=1))
        pp = ctx.enter_context(tc.tile_pool(name="pp", bufs=3))
        sm = ctx.enter_context(tc.tile_pool(name="sm", bufs=2))
        ps_sc = ctx.enter_context(tc.tile_pool(name="ps_sc", bufs=2, space="PSUM"))
        ps_a4 = ctx.enter_context(tc.tile_pool(name="ps_a4", bufs=2, space="PSUM"))
        ps_to = ctx.enter_context(tc.tile_pool(name="ps_to", bufs=2, space="PSUM"))

        # --- input DMAs ---
        w_all = const.tile([P, 8, HID], BF16, tag="w", name="w_all")
        nc.sync.dma_start(out=w_all, in_=w_d[:, :, :])
        xq_sb = big.tile([P, 2, NQ], BF16, tag="xq", name="xq_sb")
        nc.sync.dma_start(out=xq_sb, in_=xq_d[:, :, :])
        xk_sb = big.tile([P, 2, NK], BF16, tag="xk", name="xk_sb")
        nc.sync.dma_start(out=xk_sb, in_=xk_d[:, :, :])
        cm_sb = big.tile([P, NQC, NKT, QC], BF16, tag="cm", name="cm_sb")
        for qc in range(NQC):
            nc.sync.dma_start(out=cm_sb[:, qc], in_=cm_d[qc])

        ident = const.tile([P, P], BF16, tag="ident", name="ident")
        make_identity(nc, ident)

        qt_sb = big.tile([P, 2, NQ], BF16, tag="qt", name="qt_sb")
        kt_sb = big.tile([P, 2, NK], BF16, tag="kt", name="kt_sb")
        v_sb = big.tile([P, NKT, NHEAD, DHEAD + 1], BF16, tag="v", name="v_sb")
        nc.vector.memset(v_sb[:, :, :, DHEAD : DHEAD + 1], 1.0)

        # --- Q/K projections (bf16, accumulate over 2 contraction chunks) ---
        # QT[o, q] = sum_i wqT[i, o] * xqT[i, q]  (wq pre-scaled 1/8 on host)
        def proj_1024(w_slot, x_sb, x_off, dst):
            ps = ps_sc.tile([P, KTG, QC], F32, tag="sc", name="ps_proj")
            v = ps.rearrange("p a b -> p (a b)")
            for j in range(2):
                for ct in range(2):
                    nc.tensor.matmul(
                        v[:, j * 512 : (j + 1) * 512],
                        lhsT=w_all[:, w_slot + ct, :],
                        rhs=x_sb[:, ct, x_off + j * 512 : x_off + (j + 1) * 512],
                        start=(ct == 0),
                        stop=(ct == 1),
                    )
            nc.gpsimd.tensor_copy(dst, v)

        for t in range(2):
            # lhsT columns select the output half t
            ps = ps_sc.tile([P, KTG, QC], F32, tag="sc", name="ps_qt")
            v = ps.rearrange("p a b -> p (a b)")
            for j in range(2):
                for ct in range(2):
                    nc.tensor.matmul(
                        v[:, j * 512 : (j + 1) * 512],
                        lhsT=w_all[:, ct, t * P : (t + 1) * P],
                        rhs=xq_sb[:, ct, j * 512 : (j + 1) * 512],
                        start=(ct == 0),
                        stop=(ct == 1),
                    )
            nc.gpsimd.tensor_copy(qt_sb[:, t, :], v)

        for t in range(2):
            for kh in range(2):
                ps = ps_sc.tile([P, KTG, QC], F32, tag="sc", name="ps_kt")
                v = ps.rearrange("p a b -> p (a b)")
                for j in range(2):
                    for ct in range(2):
                        nc.tensor.matmul(
                            v[:, j * 512 : (j + 1) * 512],
                            lhsT=w_all[:, 2 + ct, t * P : (t + 1) * P],
                            rhs=xk_sb[:, ct, kh * 1024 + j * 512 : kh * 1024 + (j + 1) * 512],
                            start=(ct == 0),
                            stop=(ct == 1),
                        )
                nc.gpsimd.tensor_copy(kt_sb[:, t, kh * 1024 : (kh + 1) * 1024], v)

        def v_proj():
            # V[k, o] = sum_i xkT[i, k] * wvT[i, o]
            for kt in range(NKT):
                ps = ps_sc.tile([P, KTG, QC], F32, tag="sc", name="ps_v")
                vv = ps.rearrange("p a b -> p (a b)")[:, 0:HID]
                for ct in range(2):
                    nc.tensor.matmul(
                        vv,
                        lhsT=xk_sb[:, ct, kt * P : (kt + 1) * P],
                        rhs=w_all[:, 4 + ct, :],
                        start=(ct == 0),
                        stop=(ct == 1),
                    )
                nc.gpsimd.tensor_copy(
                    v_sb[:, kt, :, 0:DHEAD],
                    vv.rearrange("p (h d) -> p h d", h=NHEAD),
                )

        # --- pipelined attention units: u = (qc, h) ---
        NU = NQC * NHEAD
        p_tiles = {}
        a4_tiles = {}

        def scores_unit(u):
            qc, h = divmod(u, NHEAD)
            t, po = h // 2, (h % 2) * DHEAD
            pt = pp.tile([P, NKT, QC], BF16, tag="p", name=f"p{u}")
            p_tiles[u] = pt
            for g in range(NKT // KTG):
                sc = ps_sc.tile([P, KTG, QC], F32, tag="sc", name=f"sc{u}_{g}")
                for j in range(KTG):
                    kt = g * KTG + j
                    nc.tensor.matmul(
                        sc[:, j],
                        lhsT=kt_sb[po : po + DHEAD, t, kt * P : (kt + 1) * P],
                        rhs=qt_sb[po : po + DHEAD, t, qc * QC : (qc + 1) * QC],
                        start=True,
                        stop=True,
                    )
                nc.scalar.activation(pt[:, g * KTG : (g + 1) * KTG, :], sc, EXP)
            nc.vector.tensor_mul(pt, pt, cm_sb[:, qc])

        def pv_unit(u):
            qc, h = divmod(u, NHEAD)
            if h == 0:
                a4_tiles[qc] = [
                    ps_a4.tile([P, NHEAD, DHEAD + 1], F32, tag="a4", name=f"a4_{qc}_{qt}")
                    for qt in range(QC // P)
                ]
            pt = p_tiles[u]
            for qt in range(QC // P):
                a4 = a4_tiles[qc][qt]
                for kt in range(NKT):
                    nc.tensor.matmul(
                        a4[:, h, :],
                        lhsT=pt[:, kt, qt * P : (qt + 1) * P],
                        rhs=v_sb[:, kt, h, :],
                        start=(h == 0 and kt == 0),
                        stop=(h == NHEAD - 1 and kt == NKT - 1),
                        skip_group_check=not (
                            (h == 0 and kt == 0) or (h == NHEAD - 1 and kt == NKT - 1)
                        ),
                    )

        def tail(qc):
            for qt in range(QC // P):
                a4 = a4_tiles[qc][qt]
                rec = sm.tile([P, NHEAD, 1], F32, tag="rec", name=f"rec{qc}_{qt}")
                nc.vector.reciprocal(rec, a4[:, :, DHEAD : DHEAD + 1])
                anorm = sm.tile([P, NHEAD, DHEAD], BF16, tag="anorm", name=f"an{qc}_{qt}")
                for h in range(NHEAD):
                    nc.vector.tensor_scalar_mul(
                        anorm[:, h, :], a4[:, h, 0:DHEAD], rec[:, h, :]
                    )
                af = anorm.rearrange("p h d -> p (h d)")
                to = ps_to.tile([P, 512], F32, tag="to", name=f"to{qc}_{qt}")
                atts = []
                for ct in range(2):
                    tslice = to[:, ct * 64 : (ct + 1) * 64].bitcast(BF16)
                    nc.tensor.transpose(tslice, af[:, ct * P : (ct + 1) * P], ident)
                    att = sm.tile([P, P], BF16, tag="att", name=f"att{qc}_{qt}_{ct}")
                    nc.gpsimd.tensor_copy(att, tslice)
                    atts.append(att)
                for ct in range(2):
                    nc.tensor.matmul(
                        to[:, 256:512],
                        lhsT=atts[ct],
                        rhs=w_all[:, 6 + ct, :],
                        start=(ct == 0),
                        stop=(ct == 1),
                    )
                ob = sm.tile([P, HID], BF16, tag="ob", name=f"ob{qc}_{qt}")
                nc.gpsimd.tensor_copy(ob, to[:, 256:512])
                q0 = qc * QC + qt * P
                nc.sync.dma_start(out=out_d[q0 : q0 + P, :], in_=ob)

        scores_unit(0)
        v_proj()
        for u in range(1, NU):
            scores_unit(u)
            pv_unit(u - 1)
            if (u - 1) % NHEAD == NHEAD - 1:
                tail((u - 1) // NHEAD)
        pv_unit(NU - 1)
        tail(NQC - 1)

    nc.compile()
    return nc


_NC = None
_last_in_maps = None


def _get_nc(with_attn_mask: bool = False) -> bass.Bass:
    global _NC
    if _NC is None:
        _NC = build()
    return _NC


def _prep_maps(q_hidden_states, k_hidden_states, attention_mask, align_mask, Wq, Wk, Wv, Wo):
    q_hidden_states = np.asarray(q_hidden_states, np.float32)
    k_hidden_states = np.asarray(k_hidden_states, np.float32)
    attention_mask = np.asarray(attention_mask, np.float32)
    align_mask = np.asarray(align_mask)
    B, Q, _ = q_hidden_states.shape
    qh_len = Q // 2  # 1024

    # weight slab [128, 8, 256]: [wq0, wq1, wk0, wk1, wv0, wv1, wo0, wo1]
    wq = np.asarray(Wq, np.float32).T / np.float32(8.0)
    wk = np.asarray(Wk, np.float32).T
    wv = np.asarray(Wv, np.float32).T
    wo = np.asarray(Wo, np.float32).T
    slabs = []
    for w in (wq, wk, wv, wo):
        slabs.append(w[0:P, :])
        slabs.append(w[P : 2 * P, :])
    w_all = np.ascontiguousarray(np.stack(slabs, axis=1)).astype(BF)

    use_am = bool(np.any(attention_mask))
    in_maps = []
    for core in range(8):
        b, qh = divmod(core, 2)
        qsl = slice(qh * qh_len, (qh + 1) * qh_len)
        xq = (
            q_hidden_states[b, qsl]
            .T.reshape(2, P, qh_len)
            .transpose(1, 0, 2)
        )
        xk = k_hidden_states[b].T.reshape(2, P, NK).transpose(1, 0, 2)
        m = align_mask[b, :, qsl].astype(np.float32)
        if use_am:
            am = np.clip(attention_mask[b, 0, qsl, :].T, -80.0, 80.0)
            m = m * np.exp(am)
        # [k=2048, q=1024] -> [qc, p, t, q]
        cm = m.reshape(NKT, P, NQC, QC).transpose(2, 1, 0, 3)
        in_maps.append(
            {
                "xq": np.ascontiguousarray(xq).astype(BF),
                "xk": np.ascontiguousarray(xk).astype(BF),
                "w": w_all,
                "cm": np.ascontiguousarray(cm).astype(BF),
            }
        )
    return in_maps


def kernel(q_hidden_states, k_hidden_states, attention_mask, align_mask, Wq, Wk, Wv, Wo):
    from concourse.bass_utils import run_bass_kernel_spmd

    nc = _get_nc()
    in_maps = _prep_maps(
        q_hidden_states, k_hidden_states, attention_mask, align_mask, Wq, Wk, Wv, Wo
    )
    global _last_in_maps
    _last_in_maps = in_maps

    B, Q = np.asarray(q_hidden_states).shape[:2]
    qh_len = Q // 2
    res = run_bass_kernel_spmd(nc, in_maps, list(range(8))).results
    out = np.empty((B, Q, HID), np.float32)
    for core in range(8):
        b, qh = divmod(core, 2)
        out[b, qh * qh_len : (qh + 1) * qh_len] = np.asarray(res[core]["out"]).astype(
            np.float32
        )
    return out


# revision 8
# speedup vs baseline: 2.5782x; 1.0859x over previous
"""KgAdapterCrossAttention kernel for 8 trn2 NeuronCores.

Sharding: core = (batch b, query-half qh). Each core: 1024 queries x 2048 keys,
4 heads, hidden 256.

Design (cost-model-driven):
  - All matmuls in bf16 (1 cycle/row on PE vs 4 for fp32).
  - Scores computed transposed S^T [k, q] per head (matches mask layout),
    softmax without max-subtraction, denominator via a ones-column in V.
  - Masking: host folds align_mask AND exp(attention_mask) into ONE bf16
    multiplier CM[k, q]; device does P = exp(S^T) * CM  (one DVE multiply).
  - exp on the ACT engine is the bottleneck (~8.4M elems/core); everything
    else (PE matmuls, DVE mask-mul, Pool psum->sbuf copies, DMA) is sized to
    hide underneath it. Pipeline unit = (q-block of 256, head).
  - PSUM budget (8 banks): scores 2x[128,4,256] (4) + A accum 2x[128,4,65]
    (2) + transpose/output 2x[128,512] (2).
  - PV accumulates A[q,65] for all 4 heads into one psum bank as a single
    merged accumulation group; per-head normalize is a per-partition scalar
    multiply; O projection after a 128x128 PE transpose of the normalized A.
  - Outputs written bf16, converted to f32 on host.
"""

import os
import sys

import numpy as np

try:
    import concourse.bass as bass
except ImportError:
    for _p in ("/opt/trn_rl_repo", os.path.expanduser("~/.axon_site/_ro/trn_rl_repo")):
        if os.path.isdir(_p) and _p not in sys.path:
            sys.path.insert(0, _p)
    import concourse.bass as bass

import ml_dtypes
import concourse.mybir as mybir
import concourse.tile as tile
from concourse import bacc
from concourse.masks import make_identity
from contextlib import ExitStack

F32 = mybir.dt.float32
BF16 = mybir.dt.bfloat16
EXP = mybir.ActivationFunctionType.Exp
BF = ml_dtypes.bfloat16

P = 128
HID = 256
NHEAD = 4
DHEAD = 64
NQ = 1024  # queries per core
NK = 2048  # keys (full)
QC = 256  # query block
NQC = NQ // QC  # 4
NKT = NK // P  # 16
KTG = 4  # k-tiles per exp group


def build() -> bass.Bass:
    nc = bacc.Bacc()
    xq_d = nc.declare_dram_parameter("xq", [P, 2, NQ], BF16, isOutput=False)
    xk_d = nc.declare_dram_parameter("xk", [P, 2, NK], BF16, isOutput=False)
    w_d = nc.declare_dram_parameter("w", [P, 8, HID], BF16, isOutput=False)
    cm_d = nc.declare_dram_parameter("cm", [NQC, P, NKT, QC], BF16, isOutput=False)
    out_d = nc.declare_dram_parameter("out", [NQ, HID], BF16, isOutput=True)

    with tile.TileContext(nc) as tc, ExitStack() as ctx:
        const = ctx.enter_context(tc.tile_pool(name="const", bufs=1))
        big = ctx.enter_context(tc.tile_pool(name="big", bufs=1))
        pp = ctx.enter_context(tc.tile_pool(name="pp", bufs=3))
        sm = ctx.enter_context(tc.tile_pool(name="sm", bufs=2))
        ps_sc = ctx.enter_context(tc.tile_pool(name="ps_sc", bufs=2, space="PSUM"))
        ps_a4 = ctx.enter_context(tc.tile_pool(name="ps_a4", bufs=2, space="PSUM"))
        ps_to = ctx.enter_context(tc.tile_pool(name="ps_to", bufs=2, space="PSUM"))

        # --- input DMAs ---
        w_all = const.tile([P, 8, HID], BF16, tag="w", name="w_all")
        nc.sync.dma_start(out=w_all, in_=w_d[:, :, :])
        xq_sb = big.tile([P, 2, NQ], BF16, tag="xq", name="xq_sb")
        nc.sync.dma_start(out=xq_sb, in_=xq_d[:, :, :])
        xk_sb = big.tile([P, 2, NK], BF16, tag="xk", name="xk_sb")
        for kh in range(2):
            nc.sync.dma_start(
                out=xk_sb[:, :, kh * 1024 : (kh + 1) * 1024],
                in_=xk_d[:, :, kh * 1024 : (kh + 1) * 1024],
            )
        cm_sb = big.tile([P, NQC, NKT, QC], BF16, tag="cm", name="cm_sb")
        for qc in range(NQC):
            nc.sync.dma_start(out=cm_sb[:, qc], in_=cm_d[qc])

        ident = const.tile([P, P], BF16, tag="ident", name="ident")
        make_identity(nc, ident)

        qt_sb = big.tile([P, 2, NQ], BF16, tag="qt", name="qt_sb")
        kt_sb = big.tile([P, 2, NK], BF16, tag="kt", name="kt_sb")
        v_sb = big.tile([P, NKT, NHEAD, DHEAD + 1], BF16, tag="v", name="v_sb")
        nc.vector.memset(v_sb[:, :, :, DHEAD : DHEAD + 1], 1.0)

        # --- Q/K projections (bf16, accumulate over 2 contraction chunks) ---
        # QT[o, q] = sum_i wqT[i, o] * xqT[i, q]  (wq pre-scaled 1/8 on host)
        def qk_proj(t):
            ps = ps_sc.tile([P, KTG, QC], F32, tag="sc", name=f"ps_qt{t}")
            v = ps.rearrange("p a b -> p (a b)")
            for j in range(2):
                for ct in range(2):
                    nc.tensor.matmul(
                        v[:, j * 512 : (j + 1) * 512],
                        lhsT=w_all[:, ct, t * P : (t + 1) * P],
                        rhs=xq_sb[:, ct, j * 512 : (j + 1) * 512],
                        start=(ct == 0),
                        stop=(ct == 1),
                    )
            nc.gpsimd.tensor_copy(qt_sb[:, t, :], v)
            for kh in range(2):
                ps = ps_sc.tile([P, KTG, QC], F32, tag="sc", name=f"ps_kt{t}{kh}")
                v = ps.rearrange("p a b -> p (a b)")
                for j in range(2):
                    for ct in range(2):
                        nc.tensor.matmul(
                            v[:, j * 512 : (j + 1) * 512],
                            lhsT=w_all[:, 2 + ct, t * P : (t + 1) * P],
                            rhs=xk_sb[:, ct, kh * 1024 + j * 512 : kh * 1024 + (j + 1) * 512],
                            start=(ct == 0),
                            stop=(ct == 1),
                        )
                nc.gpsimd.tensor_copy(kt_sb[:, t, kh * 1024 : (kh + 1) * 1024], v)

        def v_proj():
            # V[k, o] = sum_i xkT[i, k] * wvT[i, o]; uses the a4 psum tag so
            # it does not stall the scores/exp ping-pong on the sc tag.
            for kt in range(NKT):
                ps = ps_a4.tile([P, NHEAD, DHEAD + 1], F32, tag="a4", name=f"ps_v{kt}")
                vv = ps.rearrange("p a b -> p (a b)")[:, 0:HID]
                for ct in range(2):
                    nc.tensor.matmul(
                        vv,
                        lhsT=xk_sb[:, ct, kt * P : (kt + 1) * P],
                        rhs=w_all[:, 4 + ct, :],
                        start=(ct == 0),
                        stop=(ct == 1),
                    )
                nc.gpsimd.tensor_copy(
                    v_sb[:, kt, :, 0:DHEAD],
                    vv.rearrange("p (h d) -> p h d", h=NHEAD),
                )

        # --- pipelined attention units: u = (qc, h) ---
        NU = NQC * NHEAD
        p_tiles = {}
        a4_tiles = {}

        def scores_unit(u):
            qc, h = divmod(u, NHEAD)
            t, po = h // 2, (h % 2) * DHEAD
            pt = pp.tile([P, NKT, QC], BF16, tag="p", name=f"p{u}")
            p_tiles[u] = pt
            for g in range(NKT // KTG):
                sc = ps_sc.tile([P, KTG, QC], F32, tag="sc", name=f"sc{u}_{g}")
                for j in range(KTG):
                    kt = g * KTG + j
                    nc.tensor.matmul(
                        sc[:, j],
                        lhsT=kt_sb[po : po + DHEAD, t, kt * P : (kt + 1) * P],
                        rhs=qt_sb[po : po + DHEAD, t, qc * QC : (qc + 1) * QC],
                        start=True,
                        stop=True,
                    )
                gs = slice(g * KTG, (g + 1) * KTG)
                nc.scalar.activation(pt[:, gs, :], sc, EXP)
                nc.vector.tensor_mul(pt[:, gs, :], pt[:, gs, :], cm_sb[:, qc, gs, :])

        def pv_unit(u):
            qc, h = divmod(u, NHEAD)
            if h == 0:
                a4_tiles[qc] = [
                    ps_a4.tile([P, NHEAD, DHEAD + 1], F32, tag="a4", name=f"a4_{qc}_{qt}")
                    for qt in range(QC // P)
                ]
            pt = p_tiles[u]
            for qt in range(QC // P):
                a4 = a4_tiles[qc][qt]
                for kt in range(NKT):
                    nc.tensor.matmul(
                        a4[:, h, :],
                        lhsT=pt[:, kt, qt * P : (qt + 1) * P],
                        rhs=v_sb[:, kt, h, :],
                        start=(h == 0 and kt == 0),
                        stop=(h == NHEAD - 1 and kt == NKT - 1),
                        skip_group_check=not (
                            (h == 0 and kt == 0) or (h == NHEAD - 1 and kt == NKT - 1)
                        ),
                    )

        def tail(qc):
            for qt in range(QC // P):
                a4 = a4_tiles[qc][qt]
                rec = sm.tile([P, NHEAD, 1], F32, tag="rec", name=f"rec{qc}_{qt}")
                nc.vector.reciprocal(rec, a4[:, :, DHEAD : DHEAD + 1])
                anorm = sm.tile([P, NHEAD, DHEAD], BF16, tag="anorm", name=f"an{qc}_{qt}")
                for h in range(NHEAD):
                    nc.vector.tensor_scalar_mul(
                        anorm[:, h, :], a4[:, h, 0:DHEAD], rec[:, h, :]
                    )
                af = anorm.rearrange("p h d -> p (h d)")
                to = ps_to.tile([P, 512], F32, tag="to", name=f"to{qc}_{qt}")
                atts = []
                for ct in range(2):
                    tslice = to[:, ct * 64 : (ct + 1) * 64].bitcast(BF16)
                    nc.tensor.transpose(tslice, af[:, ct * P : (ct + 1) * P], ident)
                    att = sm.tile([P, P], BF16, tag="att", name=f"att{qc}_{qt}_{ct}")
                    nc.gpsimd.tensor_copy(att, tslice)
                    atts.append(att)
                for ct in range(2):
                    nc.tensor.matmul(
                        to[:, 256:512],
                        lhsT=atts[ct],
                        rhs=w_all[:, 6 + ct, :],
                        start=(ct == 0),
                        stop=(ct == 1),
                    )
                ob = sm.tile([P, HID], BF16, tag="ob", name=f"ob{qc}_{qt}")
                nc.gpsimd.tensor_copy(ob, to[:, 256:512])
                q0 = qc * QC + qt * P
                nc.sync.dma_start(out=out_d[q0 : q0 + P, :], in_=ob)

        qk_proj(0)
        scores_unit(0)
        qk_proj(1)
        scores_unit(1)
        v_proj()
        pv_unit(0)
        for u in range(2, NU):
            scores_unit(u)
            pv_unit(u - 1)
            if (u - 1) % NHEAD == NHEAD - 1:
                tail((u - 1) // NHEAD)
        pv_unit(NU - 1)
        tail(NQC - 1)

    nc.compile()
    return nc


_NC = None
_last_in_maps = None


def _get_nc(with_attn_mask: bool = False) -> bass.Bass:
    global _NC
    if _NC is None:
        _NC = build()
    return _NC


def _prep_maps(q_hidden_states, k_hidden_states, attention_mask, align_mask, Wq, Wk, Wv, Wo):
    q_hidden_states = np.asarray(q_hidden_states, np.float32)
    k_hidden_states = np.asarray(k_hidden_states, np.float32)
    attention_mask = np.asarray(attention_mask, np.float32)
    align_mask = np.asarray(align_mask)
    B, Q, _ = q_hidden_states.shape
    qh_len = Q // 2  # 1024

    # weight slab [128, 8, 256]: [wq0, wq1, wk0, wk1, wv0, wv1, wo0, wo1]
    wq = np.asarray(Wq, np.float32).T / np.float32(8.0)
    wk = np.asarray(Wk, np.float32).T
    wv = np.asarray(Wv, np.float32).T
    wo = np.asarray(Wo, np.float32).T
    slabs = []
    for w in (wq, wk, wv, wo):
        slabs.append(w[0:P, :])
        slabs.append(w[P : 2 * P, :])
    w_all = np.ascontiguousarray(np.stack(slabs, axis=1)).astype(BF)

    use_am = bool(np.any(attention_mask))
    in_maps = []
    for core in range(8):
        b, qh = divmod(core, 2)
        qsl = slice(qh * qh_len, (qh + 1) * qh_len)
        xq = (
            q_hidden_states[b, qsl]
            .T.reshape(2, P, qh_len)
            .transpose(1, 0, 2)
        )
        xk = k_hidden_states[b].T.reshape(2, P, NK).transpose(1, 0, 2)
        m = align_mask[b, :, qsl].astype(np.float32)
        if use_am:
            am = np.clip(attention_mask[b, 0, qsl, :].T, -80.0, 80.0)
            m = m * np.exp(am)
        # [k=2048, q=1024] -> [qc, p, t, q]
        cm = m.reshape(NKT, P, NQC, QC).transpose(2, 1, 0, 3)
        in_maps.append(
            {
                "xq": np.ascontiguousarray(xq).astype(BF),
                "xk": np.ascontiguousarray(xk).astype(BF),
                "w": w_all,
                "cm": np.ascontiguousarray(cm).astype(BF),
            }
        )
    return in_maps


def kernel(q_hidden_states, k_hidden_states, attention_mask, align_mask, Wq, Wk, Wv, Wo):
    from concourse.bass_utils import run_bass_kernel_spmd

    nc = _get_nc()
    in_maps = _prep_maps(
        q_hidden_states, k_hidden_states, attention_mask, align_mask, Wq, Wk, Wv, Wo
    )
    global _last_in_maps
    _last_in_maps = in_maps

    B, Q = np.asarray(q_hidden_states).shape[:2]
    qh_len = Q // 2
    res = run_bass_kernel_spmd(nc, in_maps, list(range(8))).results
    out = np.empty((B, Q, HID), np.float32)
    for core in range(8):
        b, qh = divmod(core, 2)
        out[b, qh * qh_len : (qh + 1) * qh_len] = np.asarray(res[core]["out"]).astype(
            np.float32
        )
    return out


# revision 26
# speedup vs baseline: 2.9272x; 1.1353x over previous
"""KgAdapterCrossAttention kernel for 8 trn2 NeuronCores.

Sharding: core = (batch b, query-half qh). Each core: 1024 queries x 2048 keys,
4 heads, hidden 256.

Design (cost-model-driven):
  - All matmuls in bf16 (1 cycle/row on PE vs 4 for fp32).
  - Scores computed transposed S^T [k, q] per head (matches mask layout),
    softmax without max-subtraction, denominator via a ones-column in V.
  - Masking: host folds align_mask AND exp(attention_mask) into ONE bf16
    multiplier CM[k, q]; device does P = exp(S^T) * CM  (one DVE multiply).
  - exp of 8.4M scores/core is the scarce resource: 3 of 4 k-tile groups per
    unit go through the ACT engine's exp table; the 4th uses a Schraudolph
    bit-trick exp on DVE (i16 = trunc(A*s + B), bitcast to bf16, ~1.8% rms on
    25% of keys) so ACT, DVE, Pool and PE all run ~50us balanced.
    Pipeline unit = (q-block of 256, head); GPSIMD never touches PSUM (hw
    rule), so it handles SBUF-side mask multiplies while DVE drains psum.
  - PSUM budget (8 banks): scores 3x[128,4,256] (6) + A/transpose/O-proj
    shared banks 2x[128,512] (2); PV accumulates A[q,65] for all 4 heads in
    one bank as one merged accumulation group; per-head normalize is a
    per-partition scalar multiply; O projection after a 128x128 PE transpose
    of the normalized A.  Outputs written bf16, converted to f32 on host.
"""

import os
import sys

import numpy as np

try:
    import concourse.bass as bass
except ImportError:
    for _p in ("/opt/trn_rl_repo", os.path.expanduser("~/.axon_site/_ro/trn_rl_repo")):
        if os.path.isdir(_p) and _p not in sys.path:
            sys.path.insert(0, _p)
    import concourse.bass as bass

import ml_dtypes
import concourse.mybir as mybir
import concourse.tile as tile
from concourse import bacc
from concourse.masks import make_identity
from contextlib import ExitStack

F32 = mybir.dt.float32
BF16 = mybir.dt.bfloat16
EXP = mybir.ActivationFunctionType.Exp
COPY = mybir.ActivationFunctionType.Copy
MULT = mybir.AluOpType.mult
ADD = mybir.AluOpType.add
I16 = mybir.dt.int16
# Schraudolph fast-exp in bf16 bit domain: i16 = trunc(A*s + B); bitcast->bf16
SCH_A = 184.6649652337873  # 2^7 / ln 2
SCH_B = 16249.0  # 127*2^7 - C + 0.5,  C = 7.5
BF = ml_dtypes.bfloat16

P = 128
HID = 256
NHEAD = 4
DHEAD = 64
NQ = 1024  # queries per core
NK = 2048  # keys (full)
QC = 256  # query block
NQC = NQ // QC  # 4
NKT = NK // P  # 16
KTGROUPS = [(0, 4), (4, 4), (8, 4), (12, 4)]  # (start, len) k-tile groups per unit
SCW = 4  # scores psum tile width (k-tiles): 2 banks


def build() -> bass.Bass:
    nc = bacc.Bacc()
    xq_d = nc.declare_dram_parameter("xq", [P, 2, NQ], BF16, isOutput=False)
    xk_d = nc.declare_dram_parameter("xk", [P, 2, NK], BF16, isOutput=False)
    w_d = nc.declare_dram_parameter("w", [P, 8, HID], BF16, isOutput=False)
    cm_d = nc.declare_dram_parameter("cm", [NQC, P, NKT, QC], BF16, isOutput=False)
    out_d = nc.declare_dram_parameter("out", [NQ, HID], BF16, isOutput=True)

    with tile.TileContext(nc) as tc, ExitStack() as ctx:
        const = ctx.enter_context(tc.tile_pool(name="const", bufs=1))
        big = ctx.enter_context(tc.tile_pool(name="big", bufs=1))
        pp = ctx.enter_context(tc.tile_pool(name="pp", bufs=4))
        sm = ctx.enter_context(tc.tile_pool(name="sm", bufs=3))
        ps_sc = ctx.enter_context(tc.tile_pool(name="ps_sc", bufs=3, space="PSUM"))
        ps_a4 = ctx.enter_context(tc.tile_pool(name="ps_a4", bufs=2, space="PSUM"))

        # --- input DMAs ---
        w_all = const.tile([P, 8, HID], BF16, tag="w", name="w_all")
        nc.sync.dma_start(out=w_all, in_=w_d[:, :, :])
        xq_sb = big.tile([P, 2, NQ], BF16, tag="xq", name="xq_sb")
        xk_sb = big.tile([P, 2, NK], BF16, tag="xk", name="xk_sb")
        nc.sync.dma_start(out=xk_sb[:, :, 0:1024], in_=xk_d[:, :, 0:1024])
        nc.sync.dma_start(out=xq_sb, in_=xq_d[:, :, :])
        nc.sync.dma_start(out=xk_sb[:, :, 1024:2048], in_=xk_d[:, :, 1024:2048])
        cm_sb = big.tile([P, NQC, NKT, QC], BF16, tag="cm", name="cm_sb")
        for qc in range(NQC):
            nc.sync.dma_start(out=cm_sb[:, qc], in_=cm_d[qc])

        ident = const.tile([P, P], BF16, tag="ident", name="ident")
        make_identity(nc, ident)

        qt_sb = big.tile([P, 2, NQ], BF16, tag="qt", name="qt_sb")
        kt_sb = big.tile([P, 2, NK], BF16, tag="kt", name="kt_sb")
        v_sb = big.tile([P, NKT, NHEAD, DHEAD + 1], BF16, tag="v", name="v_sb")
        nc.vector.memset(v_sb[:, :, :, DHEAD : DHEAD + 1], 1.0)

        class _ActCopy:
            @staticmethod
            def tensor_copy(out, in_):
                nc.scalar.activation(out, in_, COPY)

        act_copy = _ActCopy()

        # --- Q/K projections (bf16, accumulate over 2 contraction chunks) ---
        # QT[o, q] = sum_i wqT[i, o] * xqT[i, q]  (wq pre-scaled 1/8 on host)
        def qk_proj(t):
            ps = ps_sc.tile([P, SCW, QC], F32, tag="sc", name=f"ps_qt{t}")
            v = ps.rearrange("p a b -> p (a b)")
            for j in range(2):
                for ct in range(2):
                    nc.tensor.matmul(
                        v[:, j * 512 : (j + 1) * 512],
                        lhsT=w_all[:, ct, t * P : (t + 1) * P],
                        rhs=xq_sb[:, ct, j * 512 : (j + 1) * 512],
                        start=(ct == 0),
                        stop=(ct == 1),
                    )
            nc.gpsimd.tensor_copy(qt_sb[:, t, :], v)
            for kh in range(2):
                ps = ps_sc.tile([P, SCW, QC], F32, tag="sc", name=f"ps_kt{t}{kh}")
                v = ps.rearrange("p a b -> p (a b)")
                for j in range(2):
                    for ct in range(2):
                        nc.tensor.matmul(
                            v[:, j * 512 : (j + 1) * 512],
                            lhsT=w_all[:, 2 + ct, t * P : (t + 1) * P],
                            rhs=xk_sb[:, ct, kh * 1024 + j * 512 : kh * 1024 + (j + 1) * 512],
                            start=(ct == 0),
                            stop=(ct == 1),
                        )
                nc.gpsimd.tensor_copy(kt_sb[:, t, kh * 1024 : (kh + 1) * 1024], v)

        def v_proj():
            # V[k, o] = sum_i xkT[i, k] * wvT[i, o]; uses the a4 psum tag so
            # it does not stall the scores/exp ping-pong on the sc tag.
            for kt in range(NKT):
                ps = ps_a4.tile([P, 512], F32, tag="a4", name=f"ps_v{kt}")
                vv = ps[:, 0:HID]
                for ct in range(2):
                    nc.tensor.matmul(
                        vv,
                        lhsT=xk_sb[:, ct, kt * P : (kt + 1) * P],
                        rhs=w_all[:, 4 + ct, :],
                        start=(ct == 0),
                        stop=(ct == 1),
                    )
                nc.vector.tensor_copy(
                    v_sb[:, kt, :, 0:DHEAD],
                    vv.rearrange("p (h d) -> p h d", h=NHEAD),
                )

        # --- pipelined attention units: u = (qc, h) ---
        NU = NQC * NHEAD
        p_tiles = {}
        a4_tiles = {}

        def scores_group(u, g0, glen):
            qc, h = divmod(u, NHEAD)
            t, po = h // 2, (h % 2) * DHEAD
            pt = p_tiles[u]
            sc = ps_sc.tile([P, SCW, QC], F32, tag="sc", name=f"sc{u}_{g0}")
            for j in range(glen):
                kt = g0 + j
                nc.tensor.matmul(
                    sc[:, j],
                    lhsT=kt_sb[po : po + DHEAD, t, kt * P : (kt + 1) * P],
                    rhs=qt_sb[po : po + DHEAD, t, qc * QC : (qc + 1) * QC],
                    start=True,
                    stop=True,
                )
            gs = slice(g0, g0 + glen)
            if g0 == 0:
                # Pool fast-exp (Schraudolph); mask-mul on DVE from the bitcast
                sch = sm.tile([P, KTGROUPS[0][1], QC], I16, tag="sch", name=f"sch{u}")
                nc.vector.tensor_scalar(sch, sc[:, 0:glen, :], SCH_A, SCH_B, MULT, ADD)
                nc.gpsimd.tensor_mul(
                    pt[:, gs, :], sch.bitcast(BF16), cm_sb[:, qc, gs, :]
                )
            else:
                nc.scalar.activation(pt[:, gs, :], sc[:, 0:glen, :], EXP)
                eng = nc.gpsimd if g0 < 12 else nc.vector
                eng.tensor_mul(pt[:, gs, :], pt[:, gs, :], cm_sb[:, qc, gs, :])

        def scores_unit(u, groups=KTGROUPS):
            pt = pp.tile([P, NKT, QC], BF16, tag="p", name=f"p{u}")
            p_tiles[u] = pt
            for g0, glen in groups:
                scores_group(u, g0, glen)

        def pv_unit(u):
            qc, h = divmod(u, NHEAD)
            if h == 0:
                a4_tiles[qc] = [
                    ps_a4.tile([P, 512], F32, tag="a4", name=f"a4_{qc}_{qt}")
                    for qt in range(QC // P)
                ]
            pt = p_tiles[u]
            for qt in range(QC // P):
                a4 = a4_tiles[qc][qt][:, 0 : NHEAD * (DHEAD + 1)].rearrange(
                    "p (h e) -> p h e", h=NHEAD
                )
                for kt in range(NKT):
                    nc.tensor.matmul(
                        a4[:, h, :],
                        lhsT=pt[:, kt, qt * P : (qt + 1) * P],
                        rhs=v_sb[:, kt, h, :],
                        start=(h == 0 and kt == 0),
                        stop=(h == NHEAD - 1 and kt == NKT - 1),
                        skip_group_check=not (
                            (h == 0 and kt == 0) or (h == NHEAD - 1 and kt == NKT - 1)
                        ),
                    )

        def tail(qc):
            for qt in range(QC // P):
                bank = a4_tiles[qc][qt]
                a4 = bank[:, 0 : NHEAD * (DHEAD + 1)].rearrange(
                    "p (h e) -> p h e", h=NHEAD
                )
                rec = sm.tile([P, NHEAD, 1], F32, tag="rec", name=f"rec{qc}_{qt}")
                nc.vector.reciprocal(rec, a4[:, :, DHEAD : DHEAD + 1])
                anorm = sm.tile([P, NHEAD, DHEAD], BF16, tag="anorm", name=f"an{qc}_{qt}")
                for h in range(NHEAD):
                    nc.vector.tensor_scalar_mul(
                        anorm[:, h, :], a4[:, h, 0:DHEAD], rec[:, h, :]
                    )
                af = anorm.rearrange("p h d -> p (h d)")
                atts = []
                for ct in range(2):
                    tslice = bank[:, 320 + ct * 64 : 320 + (ct + 1) * 64].bitcast(BF16)
                    nc.tensor.transpose(tslice, af[:, ct * P : (ct + 1) * P], ident)
                    att = sm.tile([P, P], BF16, tag="att", name=f"att{qc}_{qt}_{ct}")
                    nc.vector.tensor_copy(att, tslice)
                    atts.append(att)
                for ct in range(2):
                    nc.tensor.matmul(
                        bank[:, 0:HID],
                        lhsT=atts[ct],
                        rhs=w_all[:, 6 + ct, :],
                        start=(ct == 0),
                        stop=(ct == 1),
                    )
                ob = sm.tile([P, HID], BF16, tag="ob", name=f"ob{qc}_{qt}")
                nc.vector.tensor_copy(ob, bank[:, 0:HID])
                q0 = qc * QC + qt * P
                nc.sync.dma_start(out=out_d[q0 : q0 + P, :], in_=ob)

        kt_proj(0, 0, copy_eng=act_copy)
        qt_proj(0, copy_eng=act_copy)
        p_tiles[0] = pp.tile([P, NKT, QC], BF16, tag="p", name="p0")
        scores_group(0, *KTGROUPS[0])
        scores_group(0, *KTGROUPS[1])
        kt_proj(0, 1, copy_eng=act_copy)
        scores_group(0, *KTGROUPS[2])
        scores_group(0, *KTGROUPS[3])
        p_tiles[1] = pp.tile([P, NKT, QC], BF16, tag="p", name="p1")
        scores_group(1, *KTGROUPS[0])
        qt_proj(1)
        scores_group(1, *KTGROUPS[1])
        kt_proj(1, 0)
        scores_group(1, *KTGROUPS[2])
        kt_proj(1, 1)
        scores_group(1, *KTGROUPS[3])
        v_proj()
        pv_unit(0)
        for u in range(2, NU):
            scores_unit(u)
            pv_unit(u - 1)
            if (u - 1) % NHEAD == NHEAD - 1:
                tail((u - 1) // NHEAD)
        pv_unit(NU - 1)
        tail(NQC - 1)

    nc.compile()
    return nc


_NC = None
_last_in_maps = None


def _get_nc(with_attn_mask: bool = False) -> bass.Bass:
    global _NC
    if _NC is None:
        _NC = build()
    return _NC


def _prep_maps(q_hidden_states, k_hidden_states, attention_mask, align_mask, Wq, Wk, Wv, Wo):
    q_hidden_states = np.asarray(q_hidden_states, np.float32)
    k_hidden_states = np.asarray(k_hidden_states, np.float32)
    attention_mask = np.asarray(attention_mask, np.float32)
    align_mask = np.asarray(align_mask)
    B, Q, _ = q_hidden_states.shape
    qh_len = Q // 2  # 1024

    # weight slab [128, 8, 256]: [wq0, wq1, wk0, wk1, wv0, wv1, wo0, wo1]
    wq = np.asarray(Wq, np.float32).T / np.float32(8.0)
    wk = np.asarray(Wk, np.float32).T
    wv = np.asarray(Wv, np.float32).T
    wo = np.asarray(Wo, np.float32).T
    slabs = []
    for w in (wq, wk, wv, wo):
        slabs.append(w[0:P, :])
        slabs.append(w[P : 2 * P, :])
    w_all = np.ascontiguousarray(np.stack(slabs, axis=1)).astype(BF)

    use_am = bool(np.any(attention_mask))
    in_maps = []
    for core in range(8):
        b, qh = divmod(core, 2)
        qsl = slice(qh * qh_len, (qh + 1) * qh_len)
        xq = (
            q_hidden_states[b, qsl]
            .T.reshape(2, P, qh_len)
            .transpose(1, 0, 2)
        )
        xk = k_hidden_states[b].T.reshape(2, P, NK).transpose(1, 0, 2)
        m = align_mask[b, :, qsl].astype(np.float32)
        if use_am:
            am = np.clip(attention_mask[b, 0, qsl, :].T, -80.0, 80.0)
            m = m * np.exp(am)
        # [k=2048, q=1024] -> [qc, p, t, q]
        cm = m.reshape(NKT, P, NQC, QC).transpose(2, 1, 0, 3)
        in_maps.append(
            {
                "xq": np.ascontiguousarray(xq).astype(BF),
                "xk": np.ascontiguousarray(xk).astype(BF),
                "w": w_all,
                "cm": np.ascontiguousarray(cm).astype(BF),
            }
        )
    return in_maps


def kernel(q_hidden_states, k_hidden_states, attention_mask, align_mask, Wq, Wk, Wv, Wo):
    from concourse.bass_utils import run_bass_kernel_spmd

    nc = _get_nc()
    in_maps = _prep_maps(
        q_hidden_states, k_hidden_states, attention_mask, align_mask, Wq, Wk, Wv, Wo
    )
    global _last_in_maps
    _last_in_maps = in_maps

    B, Q = np.asarray(q_hidden_states).shape[:2]
    qh_len = Q // 2
    res = run_bass_kernel_spmd(nc, in_maps, list(range(8))).results
    out = np.empty((B, Q, HID), np.float32)
    for core in range(8):
        b, qh = divmod(core, 2)
        out[b, qh * qh_len : (qh + 1) * qh_len] = np.asarray(res[core]["out"]).astype(
            np.float32
        )
    return out
